# revision 3
# baseline (speedup 1.0000x reference)
"""LoRA attention (B=32, N=577, C=768, H=12, d=64, R=64) on 8 TRN2 cores.

Data-parallel over batch (4 images/core, weights replicated). Channel-major
activations; whole chain on PE in bf16 (fp32 PSUM accumulate): QKV,
S_T = K_h^T Q_h, exp, O_T = V_aug^T E, proj. Softmax denominator comes from
a ones-augmented V column; denom row is staged to SBUF by ACT (copy shares
the exp table), reciprocated with the 1-instr approx DVE op, broadcast
across 64 partitions on the idle GPSIMD/Pool engine, and multiplied in at
PSUM eviction. q-splits are [512, 65] so every PSUM target starts at a bank
boundary inside a 2-bank pair tile (one MM accumulation group per bank) and
every eviction/exp is ONE cross-bank instruction over [.., 0:577].
PSUM: 2 "ops" pair-tiles + 2 "sc" pair-tiles = 8 banks.
"""

import contextlib

import numpy as np
import ml_dtypes

import concourse.bacc as bacc
import concourse.mybir as mybir
import concourse.tile as tile
from concourse.bass_utils import run_bass_kernel_spmd

NCORES = 8
B, N, C = 32, 577, 768
H, D, R = 12, 64, 64
BC = B // NCORES    # batches per core
KT = C // 128       # 6 k-tiles over channels
NT = 5              # nk tiles: 4*128 + 65
TS = [128, 128, 128, 128, 65]          # nk tile sizes
TO = [0, 128, 256, 384, 512]           # nk tile offsets
W2 = [512, 65]                         # q splits (bank-aligned in pair tile)
O2 = [0, 512]                          # q split offsets
SCALE = D ** -0.5

F32 = mybir.dt.float32
BF16 = mybir.dt.bfloat16
EXP = mybir.ActivationFunctionType.Exp


def build_program(repeat=0):
    nc = bacc.Bacc("TRN2", target_bir_lowering=False, debug=False,
                   enable_asserts=True, num_devices=NCORES)

    xt_d = nc.dram_tensor("xT", [BC, C, N], BF16, kind="ExternalInput").ap()
    wqk_d = nc.dram_tensor("w_qk", [128, 12, KT, 128], BF16, kind="ExternalInput").ap()
    wv_d = nc.dram_tensor("w_v", [128, KT, C], BF16, kind="ExternalInput").ap()
    akv_d = nc.dram_tensor("a_kv", [128, KT, 128], BF16, kind="ExternalInput").ap()
    bk_d = nc.dram_tensor("b_k", [64, KT, 128], BF16, kind="ExternalInput").ap()
    bv_d = nc.dram_tensor("b_v", [64, C], BF16, kind="ExternalInput").ap()
    wp_d = nc.dram_tensor("w_p", [128, KT, KT, 128], BF16, kind="ExternalInput").ap()
    pb_d = nc.dram_tensor("p_b", [128, KT], F32, kind="ExternalInput").ap()
    onescol_d = nc.dram_tensor("ones_col", [128, NT, H, 1], BF16, kind="ExternalInput").ap()
    yt_d = nc.dram_tensor("yT", [BC, C, N], F32, kind="ExternalOutput").ap()

    with tile.TileContext(nc) as tc:
        with (
            tc.tile_pool(name="const", bufs=1) as cpool,
            tc.tile_pool(name="xin", bufs=2) as xpool,
            tc.tile_pool(name="qk", bufs=2) as qkpool,
            tc.tile_pool(name="vau", bufs=2) as vpool,
            tc.tile_pool(name="exp", bufs=4) as epool,
            tc.tile_pool(name="onorm", bufs=2) as opool,
            tc.tile_pool(name="small", bufs=2) as smpool,
            tc.tile_pool(name="yout", bufs=2) as ypool,
            tc.tile_pool(name="ps", bufs=2, space="PSUM") as pspool,
        ):
            # --- resident weights (first-use order) ---
            akv = cpool.tile([128, KT * 128], BF16)
            nc.sync.dma_start(out=akv[:, :], in_=akv_d.rearrange("p k c -> p (k c)"))
            wqk = cpool.tile([128, 12 * KT * 128], BF16)
            for m in range(12):
                nc.sync.dma_start(out=wqk[:, m * KT * 128:(m + 1) * KT * 128],
                                  in_=wqk_d[:, m].rearrange("p k c -> p (k c)"))
            bkv = cpool.tile([128, C], BF16)
            nc.sync.dma_start(out=bkv[0:64, :], in_=bk_d.rearrange("p k c -> p (k c)"))
            nc.sync.dma_start(out=bkv[64:128, :], in_=bv_d[:, :])
            wv = cpool.tile([128, KT * C], BF16)
            for k in range(KT):
                nc.sync.dma_start(out=wv[:, k * C:(k + 1) * C], in_=wv_d[:, k])
            wp = cpool.tile([128, KT * KT * 128], BF16)
            for m in range(KT):
                nc.sync.dma_start(out=wp[:, m * KT * 128:(m + 1) * KT * 128],
                                  in_=wp_d[:, m].rearrange("p k c -> p (k c)"))
            pb = cpool.tile([128, KT], F32)
            nc.sync.dma_start(out=pb[:, :], in_=pb_d[:, :])

            loop_cm = tc.For_i(0, repeat, 1) if repeat else contextlib.nullcontext()
            with loop_cm:
                for b in range(BC):
                    # --- x^T for this batch: [128, KT, N] ---
                    xt = xpool.tile([128, KT * N], BF16, tag="xt")
                    for k in range(KT):
                        nc.sync.dma_start(
                            out=xt[:, k * N:(k + 1) * N],
                            in_=xt_d[b, k * 128:(k + 1) * 128, :],
                        )

                    # --- LoRA down: u = A_kv @ x -> [128(r_k|r_v), N] ---
                    pt = pspool.tile([128, 1024], F32, tag="sc", name="u_ps")
                    for j in range(2):
                        for k in range(KT):
                            nc.tensor.matmul(
                                pt[:, O2[j]:O2[j] + W2[j]],
                                lhsT=akv[:, k * 128:(k + 1) * 128],
                                rhs=xt[:, k * N + O2[j]: k * N + O2[j] + W2[j]],
                                start=(k == 0), stop=(k == KT - 1),
                            )
                    u = xpool.tile([128, N], BF16, tag="u")
                    nc.vector.tensor_copy(u[:, 0:N], pt[:, 0:N])

                    # --- q, k projections (+ fused LoRA delta on k) ---
                    qk = qkpool.tile([128, 12 * N], BF16, tag="qk")
                    for m in range(12):
                        pt = pspool.tile([128, 1024], F32, tag="sc", name="qk_ps")
                        for j in range(2):
                            for k in range(KT):
                                nc.tensor.matmul(
                                    pt[:, O2[j]:O2[j] + W2[j]],
                                    lhsT=wqk[:, (m * KT + k) * 128:(m * KT + k + 1) * 128],
                                    rhs=xt[:, k * N + O2[j]: k * N + O2[j] + W2[j]],
                                    start=(k == 0),
                                    stop=(k == KT - 1 and m < 6),
                                )
                            if m >= 6:  # k-head LoRA: += B_k^T-tile @ u_k
                                nc.tensor.matmul(
                                    pt[:, O2[j]:O2[j] + W2[j]],
                                    lhsT=bkv[0:64, (m - 6) * 128:(m - 5) * 128],
                                    rhs=u[0:64, O2[j]:O2[j] + W2[j]],
                                    start=False, stop=True,
                                )
                        nc.vector.tensor_copy(qk[:, m * N:(m + 1) * N], pt[:, 0:N])

                    # --- V token-major, ones-augmented: [128, nt, 12, 65] ---
                    vaug = vpool.tile([128, NT * H * 65], BF16, tag="vaug")
                    for nt in range(NT):
                        t = TS[nt]
                        pt = pspool.tile([128, 1024], F32, tag="sc", name="v_ps")
                        for oc in range(2):  # oc halves of C: 2 x 384
                            base = oc * 512
                            for k in range(KT):
                                nc.tensor.matmul(
                                    pt[:t, base:base + 384],
                                    lhsT=xt[:, k * N + TO[nt]: k * N + TO[nt] + t],
                                    rhs=wv[:, k * C + oc * 384: k * C + oc * 384 + 384],
                                    start=(k == 0), stop=False,
                                )
                            nc.tensor.matmul(  # v-head LoRA: += u_v-tile @ B_v^T
                                pt[:t, base:base + 384],
                                lhsT=u[64:128, TO[nt]: TO[nt] + t],
                                rhs=bkv[64:128, oc * 384: oc * 384 + 384],
                                start=False, stop=True,
                            )
                        dst = vaug[:t, nt * H * 65:(nt + 1) * H * 65]
                        nc.vector.tensor_copy(
                            dst.rearrange("p (o h c) -> p o h c", o=2, h=6)[:, :, :, 0:64],
                            pt[:t, :].rearrange("p (o q) -> p o q", o=2)[:, :, 0:384]
                            .rearrange("p o (h c) -> p o h c", h=6),
                        )
                    nc.sync.dma_start(
                        out=vaug[:, :].rearrange(
                            "p (t h c) -> p t h c", t=NT, h=H)[:, :, :, 64:65],
                        in_=onescol_d[:, :, :, :],
                    )

                    # --- attention per head (pair p, half h) ---
                    onorm = opool.tile([128, KT * N], BF16, tag="onorm")
                    for p in range(6):
                        for h in range(2):
                            base = h * 64
                            hh = 2 * p + h
                            ops = pspool.tile([128, 1024], F32, tag="ops",
                                              name=f"ops{h}")
                            for nt in range(NT):
                                t = TS[nt]
                                sc = pspool.tile([128, 1024], F32, tag="sc",
                                                 name="sc")
                                for j in range(2):
                                    nc.tensor.matmul(
                                        sc[:t, O2[j]:O2[j] + W2[j]],
                                        lhsT=qk[base:base + 64,
                                                (6 + p) * N + TO[nt]:(6 + p) * N + TO[nt] + t],
                                        rhs=qk[base:base + 64,
                                               p * N + O2[j]: p * N + O2[j] + W2[j]],
                                        start=True, stop=True,
                                    )
                                et = epool.tile([128, N], BF16, tag="e")
                                nc.scalar.activation(
                                    et[:t, 0:N], sc[:t, 0:N], EXP, scale=SCALE)
                                for j in range(2):
                                    nc.tensor.matmul(
                                        ops[0:65, O2[j]:O2[j] + W2[j]],
                                        lhsT=vaug[:t, nt * H * 65 + hh * 65: nt * H * 65 + hh * 65 + 65],
                                        rhs=et[:t, O2[j]:O2[j] + W2[j]],
                                        start=(nt == 0), stop=(nt == NT - 1),
                                    )
                            # denominator -> SBUF (ACT), recip (DVE approx),
                            # broadcast (Pool), normalize-evict (DVE)
                            dall = smpool.tile([1, N], F32, tag="dall")
                            nc.scalar.copy(dall[0:1, 0:N], ops[64:65, 0:N])
                            recip = smpool.tile([1, N], F32, tag="recip")
                            with nc.allow_low_precision(reason="softmax recip"):
                                nc.vector.reciprocal_approx_fast(
                                    out=recip[0:1, 0:N], in_=dall[0:1, 0:N])
                            bcast = smpool.tile([64, N], F32, tag="bcast")
                            nc.gpsimd.partition_broadcast(
                                bcast[:, 0:N], recip[0:1, 0:N], channels=64)
                            if h == 0:
                                nc.vector.tensor_mul(
                                    onorm[0:64, p * N:(p + 1) * N],
                                    ops[0:64, 0:N], bcast[:, 0:N])
                            else:
                                stage = smpool.tile([64, N], BF16, tag="stage")
                                nc.vector.tensor_mul(
                                    stage[:, 0:N], ops[0:64, 0:N], bcast[:, 0:N])
                                nc.sync.dma_start(
                                    out=onorm[64:128, p * N:(p + 1) * N],
                                    in_=stage[:, :])

                    # --- output projection + bias ---
                    for m in range(KT):
                        pt = pspool.tile([128, 1024], F32, tag="sc", name="pj_ps")
                        for j in range(2):
                            for k in range(KT):
                                nc.tensor.matmul(
                                    pt[:, O2[j]:O2[j] + W2[j]],
                                    lhsT=wp[:, (m * KT + k) * 128:(m * KT + k + 1) * 128],
                                    rhs=onorm[:, k * N + O2[j]: k * N + O2[j] + W2[j]],
                                    start=(k == 0), stop=(k == KT - 1),
                                )
                        yt = ypool.tile([128, N], F32, tag="y")
                        nc.vector.tensor_scalar_add(
                            yt[:, 0:N], pt[:, 0:N], pb[:, m: m + 1])
                        nc.sync.dma_start(
                            out=yt_d[b, m * 128:(m + 1) * 128, :], in_=yt[:, :])

    nc.compile()
    return nc


_NC = {}


def _get_nc(repeat=0):
    if repeat not in _NC:
        _NC[repeat] = build_program(repeat)
    return _NC[repeat]


def _ones_col():
    oc = np.zeros((128, NT, H, 1), np.float32)
    for t in range(NT):
        for p in range(128):
            if t * 128 + p < N:
                oc[p, t, :, 0] = 1.0
    return oc.astype(ml_dtypes.bfloat16)


def _prep_maps(x, qkv_w, proj_w, proj_b, lora_A_k, lora_B_k, lora_A_v, lora_B_v):
    f = np.float32
    bf = ml_dtypes.bfloat16
    x = np.asarray(x, f)
    qkv_w = np.asarray(qkv_w, f)
    proj_w = np.asarray(proj_w, f)
    proj_b = np.asarray(proj_b, f)
    A_kv = np.concatenate([np.asarray(lora_A_k, f), np.asarray(lora_A_v, f)], 0)
    B_k = np.asarray(lora_B_k, f)
    B_v = np.asarray(lora_B_v, f)

    shared = {
        "w_qk": np.ascontiguousarray(
            qkv_w[:2 * C].reshape(12, 128, KT, 128).transpose(3, 0, 2, 1)).astype(bf),
        "w_v": np.ascontiguousarray(
            qkv_w[2 * C:].T.reshape(KT, 128, C).transpose(1, 0, 2)).astype(bf),
        "a_kv": np.ascontiguousarray(
            A_kv.T.reshape(KT, 128, 128).transpose(1, 0, 2)).astype(bf),
        "b_k": np.ascontiguousarray(B_k.reshape(KT, 128, 64).transpose(2, 0, 1)).astype(bf),
        "b_v": np.ascontiguousarray(B_v.T).astype(bf),
        "w_p": np.ascontiguousarray(
            proj_w.reshape(KT, 128, KT, 128).transpose(3, 0, 2, 1)).astype(bf),
        "p_b": np.ascontiguousarray(proj_b.reshape(KT, 128).T),
        "ones_col": _ones_col(),
    }
    in_maps = []
    for c in range(NCORES):
        xc = x[c * BC:(c + 1) * BC]                       # [BC, N, C]
        in_maps.append({
            "xT": np.ascontiguousarray(xc.transpose(0, 2, 1)).astype(bf),
            **shared})
    return in_maps


def kernel(x, task, qkv_w, proj_w, proj_b, lora_A_k, lora_B_k, lora_A_v,
           lora_B_v, _trace=False, _trace_kwargs=None, _repeat=0):
    nc = _get_nc(_repeat)
    in_maps = _prep_maps(x, qkv_w, proj_w, proj_b,
                         lora_A_k, lora_B_k, lora_A_v, lora_B_v)
    res = run_bass_kernel_spmd(nc, in_maps, list(range(NCORES)),
                               trace=_trace, **(_trace_kwargs or {}))
    out = np.empty((B, N, C), np.float32)
    for c in range(NCORES):
        yT = res.results[c]["yT"]                          # [BC, C, N]
        out[c * BC:(c + 1) * BC] = yT.transpose(0, 2, 1)
    if _trace:
        return out, res
    return out
